# revision 19
# baseline (speedup 1.0000x reference)
"""Causal GQA self-attention (B=2,T=2048,C=4096, 32 q-heads, 8 kv-groups, hs=128)
sharded tensor-parallel across 8 TRN2 NeuronCores: one kv-group (4 q heads) per core.

v3: single dense PE stream with software-pipelined fill scheduling.
  seg1: qkv+rope for b=0 chunks (dense), v-transposes interleaved
  seg2: b=0 attention, PE bubbles filled with b=1 qkv matmul groups
  seg3: b=1 attention, filled with b=0 output-projection groups
  seg4: remaining projection, dense
Attention per (b,tcq,h): scores computed in 4-slice "quads" ([128,4,512] psum,
one exp ACTIVATE over all 2048 cols), diagonal quads use shrinking-N matmuls
(triangular), tril mask on DVE.
Softmax denominator: exp'd slices are folded elementwise into a [128,512] bf16
accumulator on GpSimd (seg2) / DVE (seg3) — engines that are otherwise idle —
then ONE ones-matmul per (b,tcq,h) reduces partitions (vs per-slice ones-
matmuls in v2: saves ~60us of PE). The den+normalize finalize is lagged one
attention group so PE never stalls on the fold chain.
Half of Wproj is prefetched into a persistent tile during late seg2 so seg3's
projection fills never wait on DMA. PE pstate is pre-ramped with dummy matmuls
on a memset tile during the startup DMA wait.
Host sums the 8 partial outputs in fp32.
"""
import math

import numpy as np
import ml_dtypes

import concourse.bass as bass
import concourse.mybir as mybir
import concourse.tile as tile
from concourse import bacc
from concourse.bass_utils import run_bass_kernel_spmd

BF16 = mybir.dt.bfloat16
F32 = mybir.dt.float32
AF = mybir.ActivationFunctionType

N_CORES = 8
B, T, C = 2, 2048, 4096
HS = 128
QPK = 4                  # q heads per kv group
GCOLS = (QPK + 2) * HS   # 768 qkv columns per group
TOK = B * T              # 4096
NCH = TOK // 512         # 8 token chunks of 512
SCALE = float(1.0 / np.sqrt(np.float32(HS)))

_NC_CACHE = None


def build_nc():
    nc = bacc.Bacc("TRN2", target_bir_lowering=False, debug=False,
                   num_devices=N_CORES)
    # host-packed layouts (see _prep_inputs)
    xt = nc.dram_tensor("xt", [128, NCH, 32, 512], BF16, kind="ExternalInput").ap()
    wq = nc.dram_tensor("wq", [128, 6, 32, 128], BF16, kind="ExternalInput").ap()
    wp = nc.dram_tensor("wp", [128, QPK, C], BF16, kind="ExternalInput").ap()
    # cos/sin slices per in-batch 512-chunk, bf16, sin pre-negated on first half
    cs = nc.dram_tensor("cs", [128, 4, 2, 512], BF16, kind="ExternalInput").ap()
    tri = nc.dram_tensor("tri", [128, 128], BF16, kind="ExternalInput").ap()
    ones = nc.dram_tensor("ones", [128, 128], BF16, kind="ExternalInput").ap()
    out = nc.dram_tensor("out", [TOK, C], BF16, kind="ExternalOutput").ap()

    with tile.TileContext(nc) as tc:
        mid_cm = tc.tile_pool(name="mid", bufs=1)
        mid = mid_cm.__enter__()
        s2_cm = tc.tile_pool(name="s2", bufs=1)
        s2 = s2_cm.__enter__()
        pf_cm = tc.tile_pool(name="pf", bufs=3, space="PSUM")
        pf = pf_cm.__enter__()
        pscr_cm = tc.tile_pool(name="pscr", bufs=1, space="PSUM")
        pscr = pscr_cm.__enter__()
        po_cm = tc.tile_pool(name="po", bufs=1, space="PSUM")
        po = po_cm.__enter__()
        s1_cm = tc.tile_pool(name="s1", bufs=1)
        s1 = s1_cm.__enter__()

        # ---- persistent sbuf ----
        qT = mid.tile([128, QPK, TOK], BF16)     # rope'd q, feature-major
        kT = mid.tile([128, TOK], BF16)          # rope'd k, feature-major
        tri_sb = mid.tile([128, 128], BF16)
        ones_sb = mid.tile([128, 128], BF16)
        wp_a = mid.tile([128, QPK, 2048], BF16)  # Wproj first half (prefetched)
        warmsrc = mid.tile([128, 512], BF16)     # zeros: PE warmup + fold seed
        v_tok = s2.tile([128, 32, 128], BF16)    # v token-major per 128-slice
        yT0 = s2.tile([128, QPK, T], BF16)       # b=0 attention out (feature-major)
        wq_sb = s1.tile([128, 6, 32, 128], BF16)

        # PE pstate pre-ramp + exp table prewarm on a zero tile (no DMA dep)
        nc.gpsimd.memset(warmsrc[:], 0)
        warm_ps = pf.tile([128, 512], F32, tag="f", name="warmps")
        for _ in range(4):
            nc.tensor.matmul(warm_ps[:], warmsrc[:, 0:128], warmsrc[:],
                             start=True, stop=True)
        warm = s2.tile([128, 128], BF16, tag="warm")
        nc.scalar.activation(warm[:], warmsrc[:, 0:128], AF.Exp, scale=1.0)

        state = {"x": {}, "cs": {}, "wp_b": None, "yT": {0: yT0}}

        def dma_chunk_piece(c, qi):
            xq = s1.tile([128, 4, 512], BF16, tag="x", bufs=14,
                         name=f"x{c}_{qi}")
            nc.sync.dma_start(xq[:], xt[:, c, qi * 4:(qi + 1) * 4, :])
            state["x"].setdefault(c, {})[qi] = xq

        def dma_chunk_cs(c):
            cst = s1.tile([128, 2, 512], BF16, tag="cs", bufs=1, name=f"cs{c}")
            nc.sync.dma_start(cst[:], cs[:, c % 4, :, :])
            state["cs"][c] = cst

        def dma_chunk(c):
            """Issue DMAs for chunk c's x (8 eighth tiles) + cos/sin."""
            for qi in range(8):
                dma_chunk_piece(c, qi)
            dma_chunk_cs(c)

        # startup: interleave wq m0/m1 pieces with x chunk-0 pieces so the
        # first k-loop can follow the DMA arrival curve.  The very first x
        # piece is split in half so the first matmul can start ~5us earlier.
        for qr in range(4):
            nc.sync.dma_start(wq_sb[:, 0, qr * 8:(qr + 1) * 8, :],
                              wq[:, 0, qr * 8:(qr + 1) * 8, :])
            if qr == 0:
                xq0 = s1.tile([128, 4, 512], BF16, tag="x", bufs=14,
                              name="x0_0")
                nc.sync.dma_start(xq0[:, 0:2, :], xt[:, 0, 0:2, :])
                nc.sync.dma_start(xq0[:, 2:4, :], xt[:, 0, 2:4, :])
                state["x"].setdefault(0, {})[0] = xq0
                dma_chunk_piece(0, 1)
            else:
                dma_chunk_piece(0, qr * 2)
                dma_chunk_piece(0, qr * 2 + 1)
            nc.sync.dma_start(wq_sb[:, 1, qr * 8:(qr + 1) * 8, :],
                              wq[:, 1, qr * 8:(qr + 1) * 8, :])
        dma_chunk_cs(0)
        # small constants not needed until seg2 — after the critical pieces
        nc.sync.dma_start(tri_sb[:], tri[:])
        nc.sync.dma_start(ones_sb[:], ones[:])
        for m in range(2, 6):
            for half in range(2):
                nc.sync.dma_start(wq_sb[:, m, half * 16:(half + 1) * 16, :],
                                  wq[:, m, half * 16:(half + 1) * 16, :])
        dma_chunk(1)

        def emit_s1_mm(c, m, kk, ps):
            xq = state["x"][c][kk // 4]
            nc.tensor.matmul(ps[:], wq_sb[:, m, kk, :], xq[:, kk % 4, :],
                             start=(kk == 0), stop=(kk == 31))

        def emit_s1_epilogue(c, m, ps):
            t0 = c * 512
            cst = state["cs"][c]
            if m == 5:                       # v: to token-major via DMA xbar
                vtmp = s1.tile([128, 512], BF16, tag="vtmp", bufs=2,
                               name=f"vt{c}")
                nc.vector.tensor_copy(vtmp[:], ps[:])
                for sb in range(4):
                    nc.sync.dma_start(v_tok[:, c * 4 + sb, :],
                                      vtmp[:, sb * 128:(sb + 1) * 128],
                                      transpose=True)
            else:                            # q heads 0-3 / k: rope
                t1 = s1.tile([128, 512], BF16, tag="t1", bufs=2, name="t1")
                nc.vector.tensor_mul(t1[:], ps[:], cst[:, 0, :])
                u = s1.tile([128, 512], BF16, tag="u", bufs=2, name="u")
                nc.vector.tensor_mul(u[0:64, :], ps[64:128, :],
                                     cst[0:64, 1, :])
                nc.vector.tensor_mul(u[64:128, :], ps[0:64, :],
                                     cst[64:128, 1, :])
                if m < 4:
                    dst = qT[:, m, t0:t0 + 512]
                else:
                    dst = kT[:, t0:t0 + 512]
                nc.vector.tensor_add(dst, t1[:], u[:])

        # fill queue: list of (kind, cycles, emit_fn)
        fillq = []

        def push_s1_chunk(c):
            if c + 1 < NCH:
                fillq.append(("dma", 0, lambda cc=c + 1: dma_chunk(cc)))
            for m in range(6):
                holder = {}
                for gi, (k0, k1) in enumerate(((0, 12), (12, 24), (24, 32))):
                    def fn(cc=c, mm=m, kk0=k0, kk1=k1, gi=gi, h=holder):
                        if gi == 0:
                            h["ps"] = pf.tile([128, 512], F32, tag="f",
                                              name=f"s1p{cc}_{mm}")
                        for kk in range(kk0, kk1):
                            emit_s1_mm(cc, mm, kk, h["ps"])
                        if kk1 == 32:
                            emit_s1_epilogue(cc, mm, h["ps"])
                    fillq.append(("s1", (k1 - k0) * 512, fn))

        def push_proj(b, tcq):
            for ccg in range(8):
                for ti in range(b * 16 + tcq * 4, b * 16 + tcq * 4 + 4):
                    def fn(t=ti, cg=ccg, bb=b):
                        if cg < 4:
                            wsrc = wp_a[:, :, cg * 512:(cg + 1) * 512]
                        else:
                            wsrc = state["wp_b"][:, :, (cg - 4) * 512:
                                                 (cg - 3) * 512]
                        yT_b = state["yT"][bb]
                        tt = t - bb * 16
                        ps_p = pf.tile([128, 512], F32, tag="f",
                                       name=f"pj{t}_{cg}")
                        for h in range(QPK):
                            nc.tensor.matmul(
                                ps_p[:], yT_b[:, h, tt * 128:(tt + 1) * 128],
                                wsrc[:, h, :],
                                start=(h == 0), stop=(h == 3))
                        ob = state["s3"].tile([128, 512], BF16, tag="ob",
                                              bufs=6, name=f"ob{t}_{cg}")
                        if (t * 8 + cg) % 2 == 0:
                            nc.vector.tensor_copy(ob[:], ps_p[:])
                        else:
                            nc.scalar.activation(ob[:], ps_p[:], AF.Copy)
                        nc.sync.dma_start(
                            out[t * 128:(t + 1) * 128,
                                cg * 512:(cg + 1) * 512], ob[:])
                    fillq.append(("pj", 4 * 512, fn))

        fill_acct = {"spent": 0, "target": 0.0}

        def emit_fill(budget, kinds):
            # cumulative accounting: unit-granularity overshoot self-corrects
            fill_acct["target"] += budget
            while (fillq and fill_acct["spent"] < fill_acct["target"]
                   and fillq[0][0] in kinds):
                kind, cyc, fn = fillq.pop(0)
                fn()
                fill_acct["spent"] += cyc

        # ---------------- attention group ----------------
        def attention_group(b, tcq, h, fill_budget, kinds, fold_eng,
                            pre=None):
            t0g = b * T + tcq * 512
            t0l = tcq * 512                      # batch-local token offset
            n_s = (tcq + 1) * 4
            ps_o = po.tile([128, 512], F32, tag="o", name=f"o{b}{tcq}{h}")
            pacc = s2.tile([128, 512], BF16, tag="pacc", bufs=2,
                           name=f"pa{b}{tcq}{h}")
            for q in range(tcq + 1):
                diag = (q == tcq)
                ps_s = pscr.tile([128, 4, 512], F32, tag="s",
                                 name=f"s{b}{tcq}{h}{q}")
                offs = []
                for j in range(4):
                    si = q * 4 + j
                    off = 128 * j if diag else 0
                    offs.append(off)
                    s0g = b * T + si * 128
                    nc.tensor.matmul(
                        ps_s[:, j, off:512], kT[:, s0g:s0g + 128],
                        qT[:, h, t0g + off:t0g + 512],
                        start=True, stop=True)
                pt = s2.tile([128, 4, 512], BF16, tag="pt", bufs=2,
                             name=f"pt{b}{tcq}{h}{q}")
                # two exp halves: v-matmuls for slices 0/1 unblock ~1us
                # before slices 2/3 finish on ACT
                nc.scalar.activation(pt[:, 0:2, :], ps_s[:, 0:2, :],
                                     AF.Exp, scale=SCALE)
                nc.scalar.activation(pt[:, 2:4, :], ps_s[:, 2:4, :],
                                     AF.Exp, scale=SCALE)
                emit_fill(fill_budget, kinds)
                if pre is not None:
                    # previous group's den+normalize: emitted before this
                    # group's v-matmuls (whose ps_o reuse waits on the
                    # previous normalize read)
                    pre()
                    pre = None
                # tril mask + denominator folds all on the fold engine
                # (GpSimd): the DVE queue then never head-of-line blocks
                # proj-psum copies behind exp-gated work
                if diag:
                    for j in range(4):
                        o = 128 * j
                        fold_eng.tensor_mul(pt[:, j, o:o + 128],
                                            pt[:, j, o:o + 128], tri_sb[:])
                for j in range(4):
                    off = offs[j]
                    if q == 0 and j == 0:
                        fold_eng.tensor_add(pacc[:], pt[:, 0, :], warmsrc[:])
                    else:
                        fold_eng.tensor_add(pacc[:, off:512],
                                            pacc[:, off:512],
                                            pt[:, j, off:512])
                for j in range(4):
                    si = q * 4 + j
                    off = offs[j]
                    nc.tensor.matmul(
                        ps_o[:, off:512], v_tok[:, b * 16 + si, :],
                        pt[:, j, off:512],
                        start=(si == 0), stop=(si == n_s - 1))

            def finalize():
                ps_d = pf.tile([128, 512], F32, tag="f", name=f"d{b}{tcq}{h}")
                nc.tensor.matmul(ps_d[:], ones_sb[:], pacc[:],
                                 start=True, stop=True)
                rden = s2.tile([128, 512], F32, tag="rd", bufs=1, name="rden")
                nc.vector.reciprocal_approx_fast(rden[:], ps_d[:])
                nc.vector.tensor_mul(state["yT"][b][:, h, t0l:t0l + 512],
                                     ps_o[:], rden[:])
            return finalize

        # ================= emission =================
        # seg1: chunks 0-3 dense (b=0 qkv).  m-tiles processed in pairs with
        # split k-halves so late-arriving x quarters get 2x the DMA lead.
        for c in range(4):
            for ma, mb in ((0, 1), (2, 3), (4, 5)):
                if ma == 0 and c + 2 < 4:
                    dma_chunk(c + 2)
                psa = pf.tile([128, 512], F32, tag="f", name=f"c{c}m{ma}")
                psb = pf.tile([128, 512], F32, tag="f", name=f"c{c}m{mb}")
                for kk in range(16):
                    emit_s1_mm(c, ma, kk, psa)
                for kk in range(16):
                    emit_s1_mm(c, mb, kk, psb)
                for kk in range(16, 32):
                    emit_s1_mm(c, ma, kk, psa)
                emit_s1_epilogue(c, ma, psa)
                for kk in range(16, 32):
                    emit_s1_mm(c, mb, kk, psb)
                emit_s1_epilogue(c, mb, psb)


        # queue b=1 qkv as fill for seg2; prefetch chunk 4 now
        dma_chunk(4)
        for c in range(4, 8):
            push_s1_chunk(c)
        # prefetch Wproj first half late in seg2 (x DMA is done by then)
        for cg in range(4):
            fillq.append(("dma", 0, lambda g=cg: nc.sync.dma_start(
                wp_a[:, :, g * 512:(g + 1) * 512],
                wp[:, :, g * 512:(g + 1) * 512])))

        s1_cycles = sum(cyc for _, cyc, _ in fillq)
        n_quads = QPK * sum(tcq + 1 for tcq in range(4))   # 40
        budget0 = s1_cycles / n_quads

        # seg2: b=0 attention + b=1 qkv fills (den fold on idle GpSimd)
        pend = None
        for tcq in range(4):
            for hh in range(QPK):
                pend = attention_group(0, tcq, hh, budget0, ("dma", "s1"),
                                       nc.gpsimd, pre=pend)
            push_proj(0, tcq)

        # drain any qkv remainder before releasing the stage-1 pool
        while fillq and fillq[0][0] in ("dma", "s1"):
            _, _, fn = fillq.pop(0)
            fn()
        if pend is not None:
            pend()
            pend = None

        # close stage-1 pool, open proj pool; Wproj second half arrives
        # during early b=1 attention (first half is already resident)
        s1_cm.__exit__(None, None, None)
        s3_cm = tc.tile_pool(name="s3", bufs=1)
        s3 = s3_cm.__enter__()
        state["s3"] = s3
        wp_b = s3.tile([128, QPK, 2048], BF16)
        state["wp_b"] = wp_b
        yT1 = s3.tile([128, QPK, T], BF16)       # b=1 attention out
        state["yT"][1] = yT1
        for cg in range(4):
            nc.sync.dma_start(wp_b[:, :, cg * 512:(cg + 1) * 512],
                              wp[:, :, (cg + 4) * 512:(cg + 5) * 512])

        pj_cycles = sum(cyc for _, cyc, _ in fillq)
        budget1 = pj_cycles / n_quads
        fill_acct["spent"] = 0
        fill_acct["target"] = 0.0

        # seg3: b=1 attention + b=0 proj fills (den fold on DVE).  tcq
        # descending: the big (1,3)/(1,2) groups self-cover their exp latency
        # while wp_b streams in; small diagonal-only groups run last.
        for tcq in range(3, -1, -1):
            for hh in range(QPK):
                pend = attention_group(1, tcq, hh, budget1, ("dma", "pj"),
                                       nc.gpsimd, pre=pend)
            push_proj(1, tcq)
        if pend is not None:
            pend()
            pend = None

        # seg4: drain remaining proj
        while fillq:
            _, _, fn = fillq.pop(0)
            fn()

        for cm in (s3_cm, po_cm, pscr_cm, pf_cm, s2_cm, mid_cm):
            cm.__exit__(None, None, None)
    nc.compile()
    return nc


def _prep_inputs(x, cos, sin, Wqkv, Wproj):
    bf = ml_dtypes.bfloat16
    # x: [B,T,C] -> xT [C, TOK] -> [128p, chunk, 32ko, 512]
    xTn = x.reshape(TOK, C).T.astype(bf)                  # [C, TOK]
    xpack = np.ascontiguousarray(
        xTn.reshape(32, 128, NCH, 512).transpose(1, 2, 0, 3))
    # cos/sin: [T, 128] -> feature-major slices [128, 4tcq, 2, 512]
    cosT = cos.T.astype(np.float32)                       # [128, T]
    sinT = sin.T.astype(np.float32)
    sinb = np.concatenate([-sinT[0:64], sinT[64:128]], axis=0)
    cspack = np.empty((128, 4, 2, 512), dtype=np.float32)
    for tc in range(4):
        cspack[:, tc, 0, :] = cosT[:, tc * 512:(tc + 1) * 512]
        cspack[:, tc, 1, :] = sinb[:, tc * 512:(tc + 1) * 512]
    cspack = cspack.astype(bf)
    p = np.arange(128)[:, None]
    f = np.arange(128)[None, :]
    tri = (p <= f).astype(bf)                             # tril mask (kv<=q)
    ones = np.ones([128, 128], dtype=bf)
    in_maps = []
    for g in range(N_CORES):
        Wg = np.ascontiguousarray(Wqkv[:, g * GCOLS:(g + 1) * GCOLS])
        # [C, 768] -> [128p, 6m, 32ko, 128]
        wqp = np.ascontiguousarray(
            Wg.reshape(32, 128, 6, 128).transpose(1, 2, 0, 3).astype(bf))
        Wpg = Wproj[g * 512:(g + 1) * 512, :]             # [512, C]
        wpp = np.ascontiguousarray(
            Wpg.reshape(QPK, 128, C).transpose(1, 0, 2).astype(bf))
        in_maps.append({
            "xt": xpack, "wq": wqp, "wp": wpp, "cs": cspack,
            "tri": tri, "ones": ones,
        })
    return in_maps


def kernel(x, cos, sin, Wqkv, Wproj, _trace=False):
    global _NC_CACHE
    x = np.asarray(x, dtype=np.float32)
    cos = np.asarray(cos, dtype=np.float32)
    sin = np.asarray(sin, dtype=np.float32)
    Wqkv = np.asarray(Wqkv, dtype=np.float32)
    Wproj = np.asarray(Wproj, dtype=np.float32)
    if _NC_CACHE is None:
        _NC_CACHE = build_nc()
    nc = _NC_CACHE
    in_maps = _prep_inputs(x, cos, sin, Wqkv, Wproj)
    res = run_bass_kernel_spmd(nc, in_maps, core_ids=list(range(N_CORES)),
                               trace=_trace)
    acc = np.zeros([TOK, C], dtype=np.float32)
    for r in res.results:
        acc += r["out"].astype(np.float32)
    if _trace:
        kernel._last_exec_ns = res.exec_time_ns
        kernel._last_trace = res.instructions_and_trace
    return acc.reshape(B, T, C)


# revision 20
# speedup vs baseline: 1.0733x; 1.0733x over previous
"""Causal GQA self-attention (B=2,T=2048,C=4096, 32 q-heads, 8 kv-groups, hs=128)
sharded tensor-parallel across 8 TRN2 NeuronCores: one kv-group (4 q heads) per core.

v3: single dense PE stream with software-pipelined fill scheduling.
  seg1: qkv+rope for b=0 chunks (dense), v-transposes interleaved
  seg2: b=0 attention, PE bubbles filled with b=1 qkv matmul groups
  seg3: b=1 attention, filled with b=0 output-projection groups
  seg4: remaining projection, dense
Attention per (b,tcq,h): scores computed in 4-slice "quads" ([128,4,512] psum,
one exp ACTIVATE over all 2048 cols), diagonal quads use shrinking-N matmuls
(triangular), tril mask on DVE.
Softmax denominator: exp'd slices are folded elementwise into a [128,512] bf16
accumulator on GpSimd (seg2) / DVE (seg3) — engines that are otherwise idle —
then ONE ones-matmul per (b,tcq,h) reduces partitions (vs per-slice ones-
matmuls in v2: saves ~60us of PE). The den+normalize finalize is lagged one
attention group so PE never stalls on the fold chain.
Half of Wproj is prefetched into a persistent tile during late seg2 so seg3's
projection fills never wait on DMA. PE pstate is pre-ramped with dummy matmuls
on a memset tile during the startup DMA wait.
Host sums the 8 partial outputs in fp32.
"""
import math

import numpy as np
import ml_dtypes

import concourse.bass as bass
import concourse.mybir as mybir
import concourse.tile as tile
from concourse import bacc
from concourse.bass_utils import run_bass_kernel_spmd

BF16 = mybir.dt.bfloat16
F32 = mybir.dt.float32
AF = mybir.ActivationFunctionType

N_CORES = 8
B, T, C = 2, 2048, 4096
HS = 128
QPK = 4                  # q heads per kv group
GCOLS = (QPK + 2) * HS   # 768 qkv columns per group
TOK = B * T              # 4096
NCH = TOK // 512         # 8 token chunks of 512
SCALE = float(1.0 / np.sqrt(np.float32(HS)))

_NC_CACHE = None


def build_nc():
    nc = bacc.Bacc("TRN2", target_bir_lowering=False, debug=False,
                   num_devices=N_CORES)
    # host-packed layouts (see _prep_inputs)
    xt = nc.dram_tensor("xt", [128, NCH, 32, 512], BF16, kind="ExternalInput").ap()
    wq = nc.dram_tensor("wq", [128, 6, 32, 128], BF16, kind="ExternalInput").ap()
    wp = nc.dram_tensor("wp", [128, QPK, C], BF16, kind="ExternalInput").ap()
    # cos/sin slices per in-batch 512-chunk, bf16, sin pre-negated on first half
    cs = nc.dram_tensor("cs", [128, 4, 2, 512], BF16, kind="ExternalInput").ap()
    tri = nc.dram_tensor("tri", [128, 128], BF16, kind="ExternalInput").ap()
    ones = nc.dram_tensor("ones", [128, 128], BF16, kind="ExternalInput").ap()
    out = nc.dram_tensor("out", [TOK, C], BF16, kind="ExternalOutput").ap()

    with tile.TileContext(nc) as tc:
        mid_cm = tc.tile_pool(name="mid", bufs=1)
        mid = mid_cm.__enter__()
        s2_cm = tc.tile_pool(name="s2", bufs=1)
        s2 = s2_cm.__enter__()
        pf_cm = tc.tile_pool(name="pf", bufs=3, space="PSUM")
        pf = pf_cm.__enter__()
        pscr_cm = tc.tile_pool(name="pscr", bufs=1, space="PSUM")
        pscr = pscr_cm.__enter__()
        po_cm = tc.tile_pool(name="po", bufs=1, space="PSUM")
        po = po_cm.__enter__()
        s1_cm = tc.tile_pool(name="s1", bufs=1)
        s1 = s1_cm.__enter__()

        # ---- persistent sbuf ----
        qT = mid.tile([128, QPK, TOK], BF16)     # rope'd q, feature-major
        kT = mid.tile([128, TOK], BF16)          # rope'd k, feature-major
        tri_sb = mid.tile([128, 128], BF16)
        ones_sb = mid.tile([128, 128], BF16)
        wp_a = mid.tile([128, QPK, 2048], BF16)  # Wproj first half (prefetched)
        warmsrc = mid.tile([128, 512], BF16)     # zeros: PE warmup + fold seed
        v_tok = s2.tile([128, 32, 128], BF16)    # v token-major per 128-slice
        yT0 = s2.tile([128, QPK, T], BF16)       # b=0 attention out (feature-major)
        wq_sb = s1.tile([128, 6, 32, 128], BF16)

        # PE pstate pre-ramp + exp table prewarm on a zero tile (no DMA dep)
        nc.gpsimd.memset(warmsrc[:], 0)
        warm_ps = pf.tile([128, 512], F32, tag="f", name="warmps")
        for _ in range(4):
            nc.tensor.matmul(warm_ps[:], warmsrc[:, 0:128], warmsrc[:],
                             start=True, stop=True)
        warm = s2.tile([128, 128], BF16, tag="warm")
        nc.scalar.activation(warm[:], warmsrc[:, 0:128], AF.Exp, scale=1.0)

        state = {"x": {}, "cs": {}, "wp_b": None, "yT": {0: yT0}}

        def dma_chunk_piece(c, qi):
            xq = s1.tile([128, 4, 512], BF16, tag="x", bufs=14,
                         name=f"x{c}_{qi}")
            nc.sync.dma_start(xq[:], xt[:, c, qi * 4:(qi + 1) * 4, :])
            state["x"].setdefault(c, {})[qi] = xq

        def dma_chunk_cs(c):
            cst = s1.tile([128, 2, 512], BF16, tag="cs", bufs=1, name=f"cs{c}")
            nc.sync.dma_start(cst[:], cs[:, c % 4, :, :])
            state["cs"][c] = cst

        def dma_chunk(c):
            """Issue DMAs for chunk c's x (8 eighth tiles) + cos/sin."""
            for qi in range(8):
                dma_chunk_piece(c, qi)
            dma_chunk_cs(c)

        # startup: interleave wq m0/m1 pieces with x chunk-0 pieces so the
        # first k-loop can follow the DMA arrival curve.  The very first x
        # piece is split in half so the first matmul can start ~5us earlier.
        for qr in range(4):
            nc.sync.dma_start(wq_sb[:, 0, qr * 8:(qr + 1) * 8, :],
                              wq[:, 0, qr * 8:(qr + 1) * 8, :])
            if qr == 0:
                xq0 = s1.tile([128, 4, 512], BF16, tag="x", bufs=14,
                              name="x0_0")
                nc.sync.dma_start(xq0[:, 0:2, :], xt[:, 0, 0:2, :])
                nc.sync.dma_start(xq0[:, 2:4, :], xt[:, 0, 2:4, :])
                state["x"].setdefault(0, {})[0] = xq0
                dma_chunk_piece(0, 1)
            else:
                dma_chunk_piece(0, qr * 2)
                dma_chunk_piece(0, qr * 2 + 1)
            nc.sync.dma_start(wq_sb[:, 1, qr * 8:(qr + 1) * 8, :],
                              wq[:, 1, qr * 8:(qr + 1) * 8, :])
        dma_chunk_cs(0)
        # small constants not needed until seg2 — after the critical pieces
        nc.sync.dma_start(tri_sb[:], tri[:])
        nc.sync.dma_start(ones_sb[:], ones[:])
        for m in range(2, 6):
            for half in range(2):
                nc.sync.dma_start(wq_sb[:, m, half * 16:(half + 1) * 16, :],
                                  wq[:, m, half * 16:(half + 1) * 16, :])
        dma_chunk(1)

        def emit_s1_mm(c, m, kk, ps):
            xq = state["x"][c][kk // 4]
            nc.tensor.matmul(ps[:], wq_sb[:, m, kk, :], xq[:, kk % 4, :],
                             start=(kk == 0), stop=(kk == 31))

        def emit_s1_epilogue(c, m, ps):
            t0 = c * 512
            cst = state["cs"][c]
            if m == 5:                       # v: to token-major via DMA xbar
                vtmp = s1.tile([128, 512], BF16, tag="vtmp", bufs=2,
                               name=f"vt{c}")
                nc.vector.tensor_copy(vtmp[:], ps[:])
                for sb in range(4):
                    nc.sync.dma_start(v_tok[:, c * 4 + sb, :],
                                      vtmp[:, sb * 128:(sb + 1) * 128],
                                      transpose=True)
            else:                            # q heads 0-3 / k: rope
                t1 = s1.tile([128, 512], BF16, tag="t1", bufs=2, name="t1")
                nc.vector.tensor_mul(t1[:], ps[:], cst[:, 0, :])
                u = s1.tile([128, 512], BF16, tag="u", bufs=2, name="u")
                nc.vector.tensor_mul(u[0:64, :], ps[64:128, :],
                                     cst[0:64, 1, :])
                nc.vector.tensor_mul(u[64:128, :], ps[0:64, :],
                                     cst[64:128, 1, :])
                if m < 4:
                    dst = qT[:, m, t0:t0 + 512]
                else:
                    dst = kT[:, t0:t0 + 512]
                nc.vector.tensor_add(dst, t1[:], u[:])

        # fill queue: list of (kind, cycles, emit_fn)
        fillq = []

        def push_s1_chunk(c):
            if c + 1 < NCH:
                fillq.append(("dma", 0, lambda cc=c + 1: dma_chunk(cc)))
            for m in range(6):
                holder = {}
                for gi, (k0, k1) in enumerate(((0, 12), (12, 24), (24, 32))):
                    def fn(cc=c, mm=m, kk0=k0, kk1=k1, gi=gi, h=holder):
                        if gi == 0:
                            h["ps"] = pf.tile([128, 512], F32, tag="f",
                                              name=f"s1p{cc}_{mm}")
                        for kk in range(kk0, kk1):
                            emit_s1_mm(cc, mm, kk, h["ps"])
                        if kk1 == 32:
                            emit_s1_epilogue(cc, mm, h["ps"])
                    fillq.append(("s1", (k1 - k0) * 512, fn))

        def push_proj(b, tcq):
            for ccg in range(8):
                for ti in range(b * 16 + tcq * 4, b * 16 + tcq * 4 + 4):
                    def fn(t=ti, cg=ccg, bb=b):
                        if cg < 4:
                            wsrc = wp_a[:, :, cg * 512:(cg + 1) * 512]
                        else:
                            wsrc = state["wp_b"][:, :, (cg - 4) * 512:
                                                 (cg - 3) * 512]
                        yT_b = state["yT"][bb]
                        tt = t - bb * 16
                        ps_p = pf.tile([128, 512], F32, tag="f",
                                       name=f"pj{t}_{cg}")
                        for h in range(QPK):
                            nc.tensor.matmul(
                                ps_p[:], yT_b[:, h, tt * 128:(tt + 1) * 128],
                                wsrc[:, h, :],
                                start=(h == 0), stop=(h == 3))
                        ob = state["s3"].tile([128, 512], BF16, tag="ob",
                                              bufs=6, name=f"ob{t}_{cg}")
                        if (t * 8 + cg) % 2 == 0:
                            nc.vector.tensor_copy(ob[:], ps_p[:])
                        else:
                            nc.scalar.activation(ob[:], ps_p[:], AF.Copy)
                        nc.sync.dma_start(
                            out[t * 128:(t + 1) * 128,
                                cg * 512:(cg + 1) * 512], ob[:])
                    fillq.append(("pj", 4 * 512, fn))

        fill_acct = {"spent": 0, "target": 0.0}

        def emit_fill(budget, kinds):
            # cumulative accounting: unit-granularity overshoot self-corrects
            fill_acct["target"] += budget
            while (fillq and fill_acct["spent"] < fill_acct["target"]
                   and fillq[0][0] in kinds):
                kind, cyc, fn = fillq.pop(0)
                fn()
                fill_acct["spent"] += cyc

        # ---------------- attention group ----------------
        def attention_group(b, tcq, h, fill_budget, kinds, fold_eng,
                            pre=None):
            t0g = b * T + tcq * 512
            t0l = tcq * 512                      # batch-local token offset
            n_s = (tcq + 1) * 4
            ps_o = po.tile([128, 512], F32, tag="o", name=f"o{b}{tcq}{h}")
            pacc = s2.tile([128, 512], BF16, tag="pacc", bufs=2,
                           name=f"pa{b}{tcq}{h}")
            for q in range(tcq + 1):
                diag = (q == tcq)
                ps_s = pscr.tile([128, 4, 512], F32, tag="s",
                                 name=f"s{b}{tcq}{h}{q}")
                offs = []
                for j in range(4):
                    si = q * 4 + j
                    off = 128 * j if diag else 0
                    offs.append(off)
                    s0g = b * T + si * 128
                    nc.tensor.matmul(
                        ps_s[:, j, off:512], kT[:, s0g:s0g + 128],
                        qT[:, h, t0g + off:t0g + 512],
                        start=True, stop=True)
                pt = s2.tile([128, 4, 512], BF16, tag="pt", bufs=2,
                             name=f"pt{b}{tcq}{h}{q}")
                nc.scalar.activation(pt[:], ps_s[:], AF.Exp, scale=SCALE)
                emit_fill(fill_budget, kinds)
                if pre is not None:
                    # previous group's den+normalize: emitted before this
                    # group's v-matmuls (whose ps_o reuse waits on the
                    # previous normalize read)
                    pre()
                    pre = None
                if diag:
                    for j in range(4):
                        o = 128 * j
                        nc.vector.tensor_mul(pt[:, j, o:o + 128],
                                             pt[:, j, o:o + 128], tri_sb[:])
                # fold exp'd slices into the denominator accumulator on a
                # non-PE engine (masked pt for diag quads).  Diagonal quads
                # fold on DVE (fast + short) so the lagged den matmul never
                # waits long on the slower GpSimd chain.
                eng = nc.vector if diag else fold_eng
                for j in range(4):
                    off = offs[j]
                    if q == 0 and j == 0:
                        eng.tensor_add(pacc[:], pt[:, 0, :], warmsrc[:])
                    else:
                        eng.tensor_add(pacc[:, off:512],
                                       pacc[:, off:512],
                                       pt[:, j, off:512])
                for j in range(4):
                    si = q * 4 + j
                    off = offs[j]
                    nc.tensor.matmul(
                        ps_o[:, off:512], v_tok[:, b * 16 + si, :],
                        pt[:, j, off:512],
                        start=(si == 0), stop=(si == n_s - 1))

            def finalize():
                ps_d = pf.tile([128, 512], F32, tag="f", name=f"d{b}{tcq}{h}")
                nc.tensor.matmul(ps_d[:], ones_sb[:], pacc[:],
                                 start=True, stop=True)
                rden = s2.tile([128, 512], F32, tag="rd", bufs=1, name="rden")
                nc.vector.reciprocal_approx_fast(rden[:], ps_d[:])
                nc.vector.tensor_mul(state["yT"][b][:, h, t0l:t0l + 512],
                                     ps_o[:], rden[:])
            return finalize

        # ================= emission =================
        # seg1: chunks 0-3 dense (b=0 qkv).  m-tiles processed in pairs with
        # split k-halves so late-arriving x quarters get 2x the DMA lead.
        for c in range(4):
            for ma, mb in ((0, 1), (2, 3), (4, 5)):
                if ma == 0 and c + 2 < 4:
                    dma_chunk(c + 2)
                psa = pf.tile([128, 512], F32, tag="f", name=f"c{c}m{ma}")
                psb = pf.tile([128, 512], F32, tag="f", name=f"c{c}m{mb}")
                for kk in range(16):
                    emit_s1_mm(c, ma, kk, psa)
                for kk in range(16):
                    emit_s1_mm(c, mb, kk, psb)
                for kk in range(16, 32):
                    emit_s1_mm(c, ma, kk, psa)
                emit_s1_epilogue(c, ma, psa)
                for kk in range(16, 32):
                    emit_s1_mm(c, mb, kk, psb)
                emit_s1_epilogue(c, mb, psb)


        # queue b=1 qkv as fill for seg2; prefetch chunk 4 now
        dma_chunk(4)
        for c in range(4, 8):
            push_s1_chunk(c)
        # prefetch Wproj first half late in seg2 (x DMA is done by then)
        for cg in range(4):
            fillq.append(("dma", 0, lambda g=cg: nc.sync.dma_start(
                wp_a[:, :, g * 512:(g + 1) * 512],
                wp[:, :, g * 512:(g + 1) * 512])))

        s1_cycles = sum(cyc for _, cyc, _ in fillq)
        n_quads = QPK * sum(tcq + 1 for tcq in range(4))   # 40
        budget0 = s1_cycles / n_quads

        # seg2: b=0 attention + b=1 qkv fills (den fold on idle GpSimd)
        pend = None
        for tcq in range(4):
            for hh in range(QPK):
                pend = attention_group(0, tcq, hh, budget0, ("dma", "s1"),
                                       nc.gpsimd, pre=pend)
            push_proj(0, tcq)

        # drain any qkv remainder before releasing the stage-1 pool
        while fillq and fillq[0][0] in ("dma", "s1"):
            _, _, fn = fillq.pop(0)
            fn()
        if pend is not None:
            pend()
            pend = None

        # close stage-1 pool, open proj pool; Wproj second half arrives
        # during early b=1 attention (first half is already resident)
        s1_cm.__exit__(None, None, None)
        s3_cm = tc.tile_pool(name="s3", bufs=1)
        s3 = s3_cm.__enter__()
        state["s3"] = s3
        wp_b = s3.tile([128, QPK, 2048], BF16)
        state["wp_b"] = wp_b
        yT1 = s3.tile([128, QPK, T], BF16)       # b=1 attention out
        state["yT"][1] = yT1
        for cg in range(4):
            nc.sync.dma_start(wp_b[:, :, cg * 512:(cg + 1) * 512],
                              wp[:, :, (cg + 4) * 512:(cg + 5) * 512])

        pj_cycles = sum(cyc for _, cyc, _ in fillq)
        budget1 = pj_cycles / n_quads
        fill_acct["spent"] = 0
        fill_acct["target"] = 0.0

        # seg3: b=1 attention + b=0 proj fills (den fold on DVE).  tcq
        # descending: the big (1,3)/(1,2) groups self-cover their exp latency
        # while wp_b streams in; small diagonal-only groups run last.
        for tcq in range(3, -1, -1):
            for hh in range(QPK):
                pend = attention_group(1, tcq, hh, budget1, ("dma", "pj"),
                                       nc.gpsimd, pre=pend)
            push_proj(1, tcq)
        if pend is not None:
            pend()
            pend = None

        # seg4: drain remaining proj
        while fillq:
            _, _, fn = fillq.pop(0)
            fn()

        for cm in (s3_cm, po_cm, pscr_cm, pf_cm, s2_cm, mid_cm):
            cm.__exit__(None, None, None)
    nc.compile()
    return nc


def _prep_inputs(x, cos, sin, Wqkv, Wproj):
    bf = ml_dtypes.bfloat16
    # x: [B,T,C] -> xT [C, TOK] -> [128p, chunk, 32ko, 512]
    xTn = x.reshape(TOK, C).T.astype(bf)                  # [C, TOK]
    xpack = np.ascontiguousarray(
        xTn.reshape(32, 128, NCH, 512).transpose(1, 2, 0, 3))
    # cos/sin: [T, 128] -> feature-major slices [128, 4tcq, 2, 512]
    cosT = cos.T.astype(np.float32)                       # [128, T]
    sinT = sin.T.astype(np.float32)
    sinb = np.concatenate([-sinT[0:64], sinT[64:128]], axis=0)
    cspack = np.empty((128, 4, 2, 512), dtype=np.float32)
    for tc in range(4):
        cspack[:, tc, 0, :] = cosT[:, tc * 512:(tc + 1) * 512]
        cspack[:, tc, 1, :] = sinb[:, tc * 512:(tc + 1) * 512]
    cspack = cspack.astype(bf)
    p = np.arange(128)[:, None]
    f = np.arange(128)[None, :]
    tri = (p <= f).astype(bf)                             # tril mask (kv<=q)
    ones = np.ones([128, 128], dtype=bf)
    in_maps = []
    for g in range(N_CORES):
        Wg = np.ascontiguousarray(Wqkv[:, g * GCOLS:(g + 1) * GCOLS])
        # [C, 768] -> [128p, 6m, 32ko, 128]
        wqp = np.ascontiguousarray(
            Wg.reshape(32, 128, 6, 128).transpose(1, 2, 0, 3).astype(bf))
        Wpg = Wproj[g * 512:(g + 1) * 512, :]             # [512, C]
        wpp = np.ascontiguousarray(
            Wpg.reshape(QPK, 128, C).transpose(1, 0, 2).astype(bf))
        in_maps.append({
            "xt": xpack, "wq": wqp, "wp": wpp, "cs": cspack,
            "tri": tri, "ones": ones,
        })
    return in_maps


def kernel(x, cos, sin, Wqkv, Wproj, _trace=False):
    global _NC_CACHE
    x = np.asarray(x, dtype=np.float32)
    cos = np.asarray(cos, dtype=np.float32)
    sin = np.asarray(sin, dtype=np.float32)
    Wqkv = np.asarray(Wqkv, dtype=np.float32)
    Wproj = np.asarray(Wproj, dtype=np.float32)
    if _NC_CACHE is None:
        _NC_CACHE = build_nc()
    nc = _NC_CACHE
    in_maps = _prep_inputs(x, cos, sin, Wqkv, Wproj)
    res = run_bass_kernel_spmd(nc, in_maps, core_ids=list(range(N_CORES)),
                               trace=_trace)
    acc = np.zeros([TOK, C], dtype=np.float32)
    for r in res.results:
        acc += r["out"].astype(np.float32)
    if _trace:
        kernel._last_exec_ns = res.exec_time_ns
        kernel._last_trace = res.instructions_and_trace
    return acc.reshape(B, T, C)
